# revision 32
# baseline (speedup 1.0000x reference)
"""NaMMAttention (multimodal video+text attention) on 8 Trainium2 cores.

Sharding: tensor-parallel over heads (16 heads -> 2 per core). Each core:
  - computes QKV projections for its 2 heads over the full sequence
    (token-major), applies QK-RMSNorm + RoPE (vid only),
  - transposes Q/K to d-major, runs attention in the S^T layout
    (softmax reduction via ones-matmul, exp without max subtraction --
    scores are bounded by |q||k|/sqrt(D) <= sqrt(128) after RMSNorm),
  - computes its partial output projection (rows of Wout owned by its
    heads) and writes a full-shape fp32 partial.
Host sums the 8 partials and adds biases.
"""

import numpy as np
import ml_dtypes

import concourse.bass as bass
import concourse.mybir as mybir
import concourse.tile as tile
from concourse import bacc
from concourse.bass_utils import run_bass_kernel_spmd
from concourse.masks import make_identity

F32 = mybir.dt.float32
BF16 = mybir.dt.bfloat16
AF = mybir.ActivationFunctionType

B, LV, LT = 2, 2048, 128
H, D = 16, 128
DIM = 2048          # VID_DIM == TXT_DIM
INNER = H * D
NCORES = 8
HPC = H // NCORES   # heads per core = 2
EPS = 1e-6
ROPE_BASE = 10000.0
SCALE = D ** -0.5

KC = DIM // 128     # 16 contraction tiles
NTV = B * LV // 128  # 32 vid token tiles
NTT = B * LT // 128  # 2 txt token tiles
NT = NTV + NTT       # 34
L = LV + LT          # 2176 per-sample seq
NKT = L // 128       # 17 k tiles per sample
NTOK = B * L         # 4352 rows of y
QBS = [(0, 512), (512, 512), (1024, 512), (1536, 512), (2048, 128)]

LAST_RESULT = None  # test harness reads exec_time_ns off this


def _t_to_skt(t):
    """token tile index -> (sample, local k-tile)"""
    if t < NTV:
        return t // 16, t % 16
    return t - NTV, 16


def build_nc():
    nc = bacc.Bacc("TRN2", target_bir_lowering=False, debug=False,
                   num_devices=NCORES)

    xt_vid = nc.dram_tensor("xt_vid", [DIM, B * LV], BF16, kind="ExternalInput").ap()
    xt_txt = nc.dram_tensor("xt_txt", [DIM, B * LT], BF16, kind="ExternalInput").ap()
    wqkv_vid = nc.dram_tensor("wqkv_vid", [DIM, 6 * 128], BF16, kind="ExternalInput").ap()
    wqkv_txt = nc.dram_tensor("wqkv_txt", [DIM, 6 * 128], BF16, kind="ExternalInput").ap()
    wout_vid = nc.dram_tensor("wout_vid", [HPC * D, DIM], BF16, kind="ExternalInput").ap()
    wout_txt = nc.dram_tensor("wout_txt", [HPC * D, DIM], BF16, kind="ExternalInput").ap()
    # rope tables with gq/gk folded in: order
    # [cosA_q, sinB_q, sinA_q, cosB_q, cosA_k, sinB_k, sinA_k, cosB_k]
    tabs = nc.dram_tensor("tabs", [4, 2, LV, 64], BF16, kind="ExternalInput").ap()
    g_txt = nc.dram_tensor("g_txt", [2, 128, HPC * D], BF16, kind="ExternalInput").ap()
    y = nc.dram_tensor("y", [NTOK, DIM], F32, kind="ExternalOutput").ap()

    with tile.TileContext(nc) as tc:
        from contextlib import ExitStack
        with ExitStack() as ctx:
            persist = ctx.enter_context(tc.tile_pool(name="persist", bufs=1))
            qT = persist.tile([128, HPC, B, L], BF16, tag="qT")
            kT = persist.tile([128, HPC, B, L], BF16, tag="kT")
            v_sb = persist.tile([128, NT, HPC * D], BF16, tag="v")
            oT = persist.tile([128, HPC, B, L], BF16, tag="oT")
            wout_sb_v = persist.tile([128, HPC, DIM], BF16, tag="wout_v")
            wout_sb_t = persist.tile([128, HPC, DIM], BF16, tag="wout_t")
            tab_sb = persist.tile([128, 4, 2, 16, 64], BF16, tag="tabs")
            g_sb = persist.tile([128, 2, HPC * D], BF16, tag="g_txt")
            ones_sb = persist.tile([128, 1], BF16, tag="ones")
            ident = persist.tile([128, 128], BF16, tag="ident")
            eps_sb = persist.tile([128, 1], F32, tag="eps")
            nc.gpsimd.memset(eps_sb[:], EPS)

            # off the critical sync-DMA queue: phase 1's first W/x loads go first
            nc.scalar.dma_start(out=wout_sb_v[:], in_=wout_vid.rearrange("(h p) n -> p h n", p=128))
            nc.scalar.dma_start(out=wout_sb_t[:], in_=wout_txt.rearrange("(h p) n -> p h n", p=128))
            nc.scalar.dma_start(out=tab_sb[:], in_=tabs.rearrange("i j (t p) f -> p i j t f", p=128))
            nc.scalar.dma_start(out=g_sb[:], in_=g_txt.rearrange("i p n -> p i n"))
            nc.gpsimd.memset(ones_sb[:], 1.0)
            make_identity(nc, ident[:])

            # ---------------- Phase 1: QKV + norm + rope + transpose -------
            with ExitStack() as p1:
                wpool = p1.enter_context(tc.tile_pool(name="wqkv", bufs=1))
                xpool = p1.enter_context(tc.tile_pool(name="xt", bufs=3))
                qkvps = p1.enter_context(tc.tile_pool(name="qkvps", bufs=3, space="PSUM"))
                trps = p1.enter_context(tc.tile_pool(name="trps", bufs=2, space="PSUM"))
                stat = p1.enter_context(tc.tile_pool(name="stat", bufs=3))
                qnp = p1.enter_context(tc.tile_pool(name="qn", bufs=2))
                qkbfp = p1.enter_context(tc.tile_pool(name="qkbf", bufs=4))
                ropep = p1.enter_context(tc.tile_pool(name="rope", bufs=3))

                pending = []

                def emit_transposes(qkbf, s, ktl):
                    for b in range(4):
                        h = b % 2
                        dst = qT if b < 2 else kT
                        pst = trps.tile([128, 128], BF16)
                        nc.tensor.transpose(pst[:], qkbf[:, b * 128:(b + 1) * 128],
                                            ident[:])
                        nc.scalar.copy(
                            dst[:, h, s, ktl * 128:(ktl + 1) * 128], pst[:])

                w_cur = None
                # txt pair first: attention k-tile 16 (txt) is needed by the
                # very first phase-2 item, so produce it early.
                pair_order = [NTV // 2] + list(range(NTV // 2))
                for pi, pair in enumerate(pair_order):
                    vid = pair < NTV // 2
                    if pi < 2:
                        w_cur = wpool.tile([128, KC, 6 * 128], BF16, tag="w")
                        src = (wqkv_vid if vid else wqkv_txt).rearrange(
                            "(kc p) n -> p kc n", p=128)
                        # per-kc chunks so the first matmuls start after 150KB,
                        # not after the whole 2.4MB weight load
                        for ci in range(KC):
                            nc.sync.dma_start(out=w_cur[:, ci, :],
                                              in_=src[:, ci, :])
                    xt_src = xt_vid if vid else xt_txt
                    col0 = pair * 256 if vid else (pair - NTV // 2) * 256
                    xt_pair = xpool.tile([128, KC, 256], BF16)
                    nc.sync.dma_start(
                        out=xt_pair[:],
                        in_=xt_src[:, col0:col0 + 256].rearrange("(kc p) t -> p kc t", p=128))

                    for tt in range(2):
                        t = 2 * pair + tt
                        s, ktl = _t_to_skt(t)
                        ps = qkvps.tile([128, 6 * 128], F32)
                        for ci in range(KC):
                            lhs = xt_pair[:, ci, tt * 128:(tt + 1) * 128]
                            nc.tensor.matmul(ps[:, 0:512], lhs, w_cur[:, ci, 0:512],
                                             start=(ci == 0), stop=(ci == KC - 1))
                            nc.tensor.matmul(ps[:, 512:768], lhs, w_cur[:, ci, 512:768],
                                             start=(ci == 0), stop=(ci == KC - 1))

                        # RMSNorm stats for q0,q1,k0,k1 (batched over the 4 blocks)
                        sqf = stat.tile([128, 512], F32, tag="sqf")
                        nc.scalar.activation(sqf[:], ps[:, 0:512], AF.Square)
                        ssum = stat.tile([128, 4], F32, tag="ssum")
                        nc.vector.tensor_reduce(
                            ssum[:], sqf[:].rearrange("p (b f) -> p b f", b=4),
                            mybir.AxisListType.X, mybir.AluOpType.add)
                        rms = stat.tile([128, 4], F32, tag="rms")
                        nc.scalar.activation(rms[:], ssum[:], AF.Sqrt,
                                             bias=eps_sb[:], scale=1.0 / D)
                        rinv = stat.tile([128, 4], F32, tag="rinv")
                        nc.vector.reciprocal(rinv[:], rms[:])
                        rinv_b = rinv[:, :, None].to_broadcast([128, 4, 128])

                        qkbf = qkbfp.tile([128, 512], BF16)
                        ps4 = ps[:, 0:512].rearrange("p (b f) -> p b f", b=4)
                        if vid:
                            qn = qnp.tile([128, 512], F32)
                            qn4 = qn[:].rearrange("p (b f) -> p b f", b=4)
                            nc.vector.tensor_tensor(qn4, ps4, rinv_b,
                                                    mybir.AluOpType.mult)
                            tt_pos = t % 16
                            # views [128, qk, h, d]
                            xv = qn[:].rearrange("p (a h f) -> p a h f", a=2, h=2)
                            qv = qkbf[:].rearrange("p (a h f) -> p a h f", a=2, h=2)
                            x1, x2 = xv[:, :, :, 0:64], xv[:, :, :, 64:128]
                            T = [tab_sb[:, i, :, tt_pos, :].unsqueeze(2)
                                 .to_broadcast([128, 2, 2, 64]) for i in range(4)]
                            tm1 = ropep.tile([128, 2, 2, 64], F32, tag="tm1")
                            tm2 = ropep.tile([128, 2, 2, 64], F32, tag="tm2")
                            nc.vector.tensor_mul(tm1[:], x1, T[0])
                            nc.vector.tensor_mul(tm2[:], x2, T[1])
                            nc.vector.tensor_sub(qv[:, :, :, 0:64], tm1[:], tm2[:])
                            tm3 = ropep.tile([128, 2, 2, 64], F32, tag="tm3")
                            tm4 = ropep.tile([128, 2, 2, 64], F32, tag="tm4")
                            nc.vector.tensor_mul(tm3[:], x1, T[2])
                            nc.vector.tensor_mul(tm4[:], x2, T[3])
                            nc.vector.tensor_add(qv[:, :, :, 64:128], tm3[:], tm4[:])
                        else:
                            qk4 = qkbf[:].rearrange("p (b f) -> p b f", b=4)
                            nc.vector.tensor_tensor(qk4, ps4, rinv_b,
                                                    mybir.AluOpType.mult)
                            for qk in range(2):
                                nc.vector.tensor_mul(qkbf[:, qk * 256:(qk + 1) * 256],
                                                     qkbf[:, qk * 256:(qk + 1) * 256],
                                                     g_sb[:, qk, :])

                        nc.scalar.copy(v_sb[:, t, :], ps[:, 512:768])
                        pending.append((qkbf, s, ktl))
                        if len(pending) > 2:
                            emit_transposes(*pending.pop(0))

                while pending:
                    emit_transposes(*pending.pop(0))

            # ------- Phase 2+3: attention + output proj, one pipeline ------
            with ExitStack() as p2:
                sps = p2.enter_context(tc.tile_pool(name="sps", bufs=2, space="PSUM"))
                dps = p2.enter_context(tc.tile_pool(name="dps", bufs=1, space="PSUM"))
                ops_ = p2.enter_context(tc.tile_pool(name="ops", bufs=2, space="PSUM"))
                yps = p2.enter_context(tc.tile_pool(name="yps", bufs=1, space="PSUM"))
                epool = p2.enter_context(tc.tile_pool(name="e", bufs=4))
                rpool = p2.enter_context(tc.tile_pool(name="rec", bufs=2))
                bpool = p2.enter_context(tc.tile_pool(name="rbc", bufs=2))
                ypool = p2.enter_context(tc.tile_pool(name="ysb", bufs=4))

                def emit_scores(s, h, qs, qn_):
                    E = epool.tile([128, NKT, 512], BF16, tag="E")
                    for j in range((NKT + 1) // 2):
                        k0 = 2 * j
                        kn = 2 if k0 + 1 < NKT else 1
                        ps_s = sps.tile([128, 1024], F32)
                        for u in range(kn):
                            nc.tensor.matmul(
                                ps_s[:, u * 512:u * 512 + qn_],
                                kT[:, h, s, (k0 + u) * 128:(k0 + u + 1) * 128],
                                qT[:, h, s, qs:qs + qn_],
                                start=True, stop=True)
                        if kn == 2 and qn_ == 512:
                            nc.scalar.activation(
                                E[:, k0:k0 + 2, :].rearrange("p a b -> p (a b)"),
                                ps_s[:], AF.Exp, scale=SCALE)
                        else:
                            for u in range(kn):
                                nc.scalar.activation(
                                    E[:, k0 + u, :qn_],
                                    ps_s[:, u * 512:u * 512 + qn_],
                                    AF.Exp, scale=SCALE)
                    return E

                def emit_tail(s, h, qs, qn_, E):
                    ps_d = dps.tile([1, 512], F32)
                    for kt in range(NKT):
                        nc.tensor.matmul(ps_d[:, :qn_], ones_sb[:],
                                         E[:, kt, :qn_],
                                         start=(kt == 0), stop=(kt == NKT - 1))
                    rec = rpool.tile([1, 512], F32)
                    nc.vector.reciprocal(rec[:, :qn_], ps_d[:, :qn_])
                    rbc = bpool.tile([128, 512], F32)
                    nc.gpsimd.partition_broadcast(rbc[:, :qn_], rec[:, :qn_])
                    ps_o = ops_.tile([128, 512], F32)
                    for kt in range(NKT):
                        vt = s * 16 + kt if kt < 16 else NTV + s
                        nc.tensor.matmul(
                            ps_o[:, :qn_],
                            v_sb[:, vt, h * 128:(h + 1) * 128],
                            E[:, kt, :qn_],
                            start=(kt == 0), stop=(kt == NKT - 1))
                    nc.vector.tensor_mul(oT[:, h, s, qs:qs + qn_],
                                         ps_o[:, :qn_], rbc[:, :qn_])

                def emit_y(s, qs, qn_):
                    for ltl in range(qs // 128, (qs + qn_) // 128):
                        t = s * 16 + ltl if ltl < 16 else NTV + s
                        wout_sb = wout_sb_v if t < NTV else wout_sb_t
                        for nb in range(4):
                            ps_y = yps.tile([128, 512], F32)
                            for hh in range(HPC):
                                nc.tensor.matmul(
                                    ps_y[:],
                                    oT[:, hh, s, ltl * 128:(ltl + 1) * 128],
                                    wout_sb[:, hh, nb * 512:(nb + 1) * 512],
                                    start=(hh == 0), stop=(hh == HPC - 1))
                            yt = ypool.tile([128, 512], F32)
                            if (t * 4 + nb) % 2 == 0:
                                nc.scalar.copy(yt[:], ps_y[:])
                            else:
                                nc.vector.tensor_copy(yt[:], ps_y[:])
                            nc.sync.dma_start(
                                out=y[t * 128:(t + 1) * 128,
                                      nb * 512:(nb + 1) * 512],
                                in_=yt[:])

                y_units = []

                def push_y(s, qs, qn_):
                    for ltl in range(qs // 128, (qs + qn_) // 128):
                        for nb in range(4):
                            y_units.append((s, ltl, nb))

                def emit_y_unit(s, ltl, nb):
                    t = s * 16 + ltl if ltl < 16 else NTV + s
                    wout_sb = wout_sb_v if t < NTV else wout_sb_t
                    ps_y = yps.tile([128, 512], F32)
                    for hh in range(HPC):
                        nc.tensor.matmul(
                            ps_y[:],
                            oT[:, hh, s, ltl * 128:(ltl + 1) * 128],
                            wout_sb[:, hh, nb * 512:(nb + 1) * 512],
                            start=(hh == 0), stop=(hh == HPC - 1))
                    yt = ypool.tile([128, 512], F32)
                    if (t * 4 + nb) % 2 == 0:
                        nc.scalar.copy(yt[:], ps_y[:])
                    else:
                        nc.vector.tensor_copy(yt[:], ps_y[:])
                    nc.sync.dma_start(
                        out=y[t * 128:(t + 1) * 128, nb * 512:(nb + 1) * 512],
                        in_=yt[:])

                NSTEP = (NKT + 1) // 2

                def finish_tail(prev):
                    (s, h, qs, qn_), E, ps_d, ps_o = prev
                    rec = rpool.tile([1, 512], F32)
                    nc.vector.reciprocal(rec[:, :qn_], ps_d[:, :qn_])
                    rbc = bpool.tile([128, 512], F32)
                    nc.gpsimd.partition_broadcast(rbc[:, :qn_], rec[:, :qn_])
                    nc.vector.tensor_mul(oT[:, h, s, qs:qs + qn_],
                                         ps_o[:, :qn_], rbc[:, :qn_])
                    if h == HPC - 1:
                        push_y(s, qs, qn_)

                items = [(s, h, qs, qn_) for s in range(B)
                         for (qs, qn_) in QBS for h in range(HPC)]
                prev = None
                for it in items:
                    s, h, qs, qn_ = it
                    E = epool.tile([128, NKT, 512], BF16, tag="E")
                    if prev is not None:
                        ps_d = dps.tile([1, 512], F32)
                        ps_o = ops_.tile([128, 512], F32)
                    for j in range(NSTEP):
                        k0 = 2 * j
                        kn = 2 if k0 + 1 < NKT else 1
                        ps_s = sps.tile([128, 1024], F32)
                        for u in range(kn):
                            nc.tensor.matmul(
                                ps_s[:, u * 512:u * 512 + qn_],
                                kT[:, h, s, (k0 + u) * 128:(k0 + u + 1) * 128],
                                qT[:, h, s, qs:qs + qn_],
                                start=True, stop=True)
                        if kn == 2 and qn_ == 512:
                            nc.scalar.activation(
                                E[:, k0:k0 + 2, :].rearrange("p a b -> p (a b)"),
                                ps_s[:], AF.Exp, scale=SCALE)
                        else:
                            for u in range(kn):
                                nc.scalar.activation(
                                    E[:, k0 + u, :qn_],
                                    ps_s[:, u * 512:u * 512 + qn_],
                                    AF.Exp, scale=SCALE)
                        if prev is not None:
                            (ss, sh, sqs, sqn), pE, _, _ = prev
                            for u in range(kn):
                                kt = k0 + u
                                nc.tensor.matmul(
                                    ps_d[:, :sqn], ones_sb[:], pE[:, kt, :sqn],
                                    start=(kt == 0), stop=(kt == NKT - 1))
                            for u in range(kn):
                                kt = k0 + u
                                vt = ss * 16 + kt if kt < 16 else NTV + ss
                                nc.tensor.matmul(
                                    ps_o[:, :sqn],
                                    v_sb[:, vt, sh * 128:(sh + 1) * 128],
                                    pE[:, kt, :sqn],
                                    start=(kt == 0), stop=(kt == NKT - 1))
                        if y_units:
                            emit_y_unit(*y_units.pop(0))
                    if prev is not None:
                        finish_tail((prev[0], prev[1], ps_d, ps_o))
                    prev = (it, E, None, None)
                # last item's tail, un-interleaved
                (s, h, qs, qn_), E, _, _ = prev
                ps_d = dps.tile([1, 512], F32)
                ps_o = ops_.tile([128, 512], F32)
                for kt in range(NKT):
                    nc.tensor.matmul(ps_d[:, :qn_], ones_sb[:], E[:, kt, :qn_],
                                     start=(kt == 0), stop=(kt == NKT - 1))
                for kt in range(NKT):
                    vt = s * 16 + kt if kt < 16 else NTV + s
                    nc.tensor.matmul(
                        ps_o[:, :qn_], v_sb[:, vt, h * 128:(h + 1) * 128],
                        E[:, kt, :qn_],
                        start=(kt == 0), stop=(kt == NKT - 1))
                finish_tail((prev[0], prev[1], ps_d, ps_o))
                while y_units:
                    emit_y_unit(*y_units.pop(0))

    nc.compile()
    return nc


_NC_CACHE = None


def _get_nc():
    global _NC_CACHE
    if _NC_CACHE is None:
        _NC_CACHE = build_nc()
    return _NC_CACHE


def _bf16(x):
    return np.ascontiguousarray(x).astype(ml_dtypes.bfloat16)


def kernel(vid, txt, vid_shape, txt_shape,
           Wqkv_vid, Wqkv_txt, gq_vid, gq_txt, gk_vid, gk_txt,
           Wout_vid, Wout_txt, bout_vid, bout_txt):
    global LAST_RESULT
    vid = np.asarray(vid, np.float32)
    txt = np.asarray(txt, np.float32)
    Wqkv_vid = np.asarray(Wqkv_vid, np.float32)
    Wqkv_txt = np.asarray(Wqkv_txt, np.float32)
    Wout_vid = np.asarray(Wout_vid, np.float32)
    Wout_txt = np.asarray(Wout_txt, np.float32)
    gq_vid = np.asarray(gq_vid, np.float32)
    gq_txt = np.asarray(gq_txt, np.float32)
    gk_vid = np.asarray(gk_vid, np.float32)
    gk_txt = np.asarray(gk_txt, np.float32)

    nc = _get_nc()

    xt_vid = _bf16(vid.T)
    xt_txt = _bf16(txt.T)

    # rope tables with g folded in
    pos = np.arange(LV, dtype=np.float32)
    inv_freq = 1.0 / (ROPE_BASE ** (np.arange(64, dtype=np.float32) / 64.0))
    ang = pos[:, None] * inv_freq[None, :]
    cos, sin = np.cos(ang), np.sin(ang)
    # tabs[i, qk]: rope factor i for {q, k}; out1 = x1*T0 - x2*T1,
    # out2 = x1*T2 + x2*T3 (g folded in)
    tabs = np.stack([
        np.stack([cos * gq_vid[None, :64], cos * gk_vid[None, :64]]),
        np.stack([sin * gq_vid[None, 64:], sin * gk_vid[None, 64:]]),
        np.stack([sin * gq_vid[None, :64], sin * gk_vid[None, :64]]),
        np.stack([cos * gq_vid[None, 64:], cos * gk_vid[None, 64:]]),
    ])
    tabs = _bf16(tabs)
    g_txt = _bf16(np.stack([
        np.tile(gq_txt, (128, HPC)),
        np.tile(gk_txt, (128, HPC)),
    ]))

    in_maps = []
    for c in range(NCORES):
        h0, h1 = HPC * c, HPC * c + 1
        def _wqkv(W):
            Wq, Wk, Wv = W[:, :INNER], W[:, INNER:2 * INNER], W[:, 2 * INNER:]
            cols = []
            for Wx in (Wq, Wk, Wv):
                cols.append(Wx[:, h0 * D:(h0 + 1) * D])
                cols.append(Wx[:, h1 * D:(h1 + 1) * D])
            return _bf16(np.concatenate(cols, axis=1))
        def _wout(W):
            return _bf16(np.concatenate(
                [W[h0 * D:(h0 + 1) * D, :], W[h1 * D:(h1 + 1) * D, :]], axis=0))
        in_maps.append({
            "xt_vid": xt_vid, "xt_txt": xt_txt,
            "wqkv_vid": _wqkv(Wqkv_vid), "wqkv_txt": _wqkv(Wqkv_txt),
            "wout_vid": _wout(Wout_vid), "wout_txt": _wout(Wout_txt),
            "tabs": tabs, "g_txt": g_txt,
        })

    try:
        res = run_bass_kernel_spmd(nc, in_maps, list(range(NCORES)))
    except Exception:
        # transient device state (e.g. NRT exec-unit wedge) — one retry
        res = run_bass_kernel_spmd(nc, in_maps, list(range(NCORES)))
    LAST_RESULT = res
    Y = np.zeros((NTOK, DIM), np.float32)
    for c in range(NCORES):
        Y += res.results[c]["y"]
    vid_out = Y[:B * LV] + np.asarray(bout_vid, np.float32)[None, :]
    txt_out = Y[B * LV:] + np.asarray(bout_txt, np.float32)[None, :]
    return vid_out, txt_out


# revision 33
# speedup vs baseline: 1.0566x; 1.0566x over previous
"""NaMMAttention (multimodal video+text attention) on 8 Trainium2 cores.

Sharding: tensor-parallel over heads (16 heads -> 2 per core). Each core:
  - computes QKV projections for its 2 heads over the full sequence
    (token-major), applies QK-RMSNorm + RoPE (vid only),
  - transposes Q/K to d-major, runs attention in the S^T layout
    (softmax reduction via ones-matmul, exp without max subtraction --
    scores are bounded by |q||k|/sqrt(D) <= sqrt(128) after RMSNorm),
  - computes its partial output projection (rows of Wout owned by its
    heads) and writes a full-shape fp32 partial.
Host sums the 8 partials and adds biases.
"""

import numpy as np
import ml_dtypes

import concourse.bass as bass
import concourse.mybir as mybir
import concourse.tile as tile
from concourse import bacc
from concourse.bass_utils import run_bass_kernel_spmd
from concourse.masks import make_identity

F32 = mybir.dt.float32
BF16 = mybir.dt.bfloat16
AF = mybir.ActivationFunctionType

B, LV, LT = 2, 2048, 128
H, D = 16, 128
DIM = 2048          # VID_DIM == TXT_DIM
INNER = H * D
NCORES = 8
HPC = H // NCORES   # heads per core = 2
EPS = 1e-6
ROPE_BASE = 10000.0
SCALE = D ** -0.5

KC = DIM // 128     # 16 contraction tiles
NTV = B * LV // 128  # 32 vid token tiles
NTT = B * LT // 128  # 2 txt token tiles
NT = NTV + NTT       # 34
L = LV + LT          # 2176 per-sample seq
NKT = L // 128       # 17 k tiles per sample
NTOK = B * L         # 4352 rows of y
QBS = [(0, 512), (512, 512), (1024, 512), (1536, 512), (2048, 128)]

LAST_RESULT = None  # test harness reads exec_time_ns off this


def _t_to_skt(t):
    """token tile index -> (sample, local k-tile)"""
    if t < NTV:
        return t // 16, t % 16
    return t - NTV, 16


def build_nc():
    nc = bacc.Bacc("TRN2", target_bir_lowering=False, debug=False,
                   num_devices=NCORES)

    xt_vid = nc.dram_tensor("xt_vid", [DIM, B * LV], BF16, kind="ExternalInput").ap()
    xt_txt = nc.dram_tensor("xt_txt", [DIM, B * LT], BF16, kind="ExternalInput").ap()
    wqkv_vid = nc.dram_tensor("wqkv_vid", [DIM, 6 * 128], BF16, kind="ExternalInput").ap()
    wqkv_txt = nc.dram_tensor("wqkv_txt", [DIM, 6 * 128], BF16, kind="ExternalInput").ap()
    wout_vid = nc.dram_tensor("wout_vid", [HPC * D, DIM], BF16, kind="ExternalInput").ap()
    wout_txt = nc.dram_tensor("wout_txt", [HPC * D, DIM], BF16, kind="ExternalInput").ap()
    # rope tables with gq/gk folded in: order
    # [cosA_q, sinB_q, sinA_q, cosB_q, cosA_k, sinB_k, sinA_k, cosB_k]
    tabs = nc.dram_tensor("tabs", [4, 2, LV, 64], BF16, kind="ExternalInput").ap()
    g_txt = nc.dram_tensor("g_txt", [2, 128, HPC * D], BF16, kind="ExternalInput").ap()
    y = nc.dram_tensor("y", [NTOK, DIM], F32, kind="ExternalOutput").ap()

    with tile.TileContext(nc) as tc:
        from contextlib import ExitStack
        with ExitStack() as ctx:
            persist = ctx.enter_context(tc.tile_pool(name="persist", bufs=1))
            qT = persist.tile([128, HPC, B, L], BF16, tag="qT")
            kT = persist.tile([128, HPC, B, L], BF16, tag="kT")
            v_sb = persist.tile([128, NT, HPC * D], BF16, tag="v")
            oT = persist.tile([128, HPC, B, L], BF16, tag="oT")
            wout_sb_v = persist.tile([128, HPC, DIM], BF16, tag="wout_v")
            wout_sb_t = persist.tile([128, HPC, DIM], BF16, tag="wout_t")
            tab_sb = persist.tile([128, 4, 2, 16, 64], BF16, tag="tabs")
            g_sb = persist.tile([128, 2, HPC * D], BF16, tag="g_txt")
            ones_sb = persist.tile([128, 1], BF16, tag="ones")
            ident = persist.tile([128, 128], BF16, tag="ident")
            eps_sb = persist.tile([128, 1], F32, tag="eps")
            nc.gpsimd.memset(eps_sb[:], EPS)

            # off the critical sync-DMA queue: phase 1's first W/x loads go first
            nc.scalar.dma_start(out=wout_sb_v[:], in_=wout_vid.rearrange("(h p) n -> p h n", p=128))
            nc.scalar.dma_start(out=wout_sb_t[:], in_=wout_txt.rearrange("(h p) n -> p h n", p=128))
            nc.scalar.dma_start(out=tab_sb[:], in_=tabs.rearrange("i j (t p) f -> p i j t f", p=128))
            nc.scalar.dma_start(out=g_sb[:], in_=g_txt.rearrange("i p n -> p i n"))
            nc.gpsimd.memset(ones_sb[:], 1.0)
            make_identity(nc, ident[:])

            # ---------------- Phase 1: QKV + norm + rope + transpose -------
            with ExitStack() as p1:
                wpool = p1.enter_context(tc.tile_pool(name="wqkv", bufs=1))
                xpool = p1.enter_context(tc.tile_pool(name="xt", bufs=3))
                qkvps = p1.enter_context(tc.tile_pool(name="qkvps", bufs=3, space="PSUM"))
                trps = p1.enter_context(tc.tile_pool(name="trps", bufs=2, space="PSUM"))
                stat = p1.enter_context(tc.tile_pool(name="stat", bufs=3))
                qnp = p1.enter_context(tc.tile_pool(name="qn", bufs=2))
                qkbfp = p1.enter_context(tc.tile_pool(name="qkbf", bufs=4))
                ropep = p1.enter_context(tc.tile_pool(name="rope", bufs=3))

                pending = []

                def emit_transposes(qkbf, s, ktl):
                    for b in range(4):
                        h = b % 2
                        dst = qT if b < 2 else kT
                        pst = trps.tile([128, 128], BF16)
                        nc.tensor.transpose(pst[:], qkbf[:, b * 128:(b + 1) * 128],
                                            ident[:])
                        nc.scalar.copy(
                            dst[:, h, s, ktl * 128:(ktl + 1) * 128], pst[:])

                w_cur = None
                # txt pair first: attention k-tile 16 (txt) is needed by the
                # very first phase-2 item, so produce it early.
                pair_order = [NTV // 2] + list(range(NTV // 2))
                for pi, pair in enumerate(pair_order):
                    vid = pair < NTV // 2
                    if pi < 2:
                        w_cur = wpool.tile([128, KC, 6 * 128], BF16, tag="w")
                        src = (wqkv_vid if vid else wqkv_txt).rearrange(
                            "(kc p) n -> p kc n", p=128)
                        # per-kc chunks so the first matmuls start after 150KB,
                        # not after the whole 2.4MB weight load
                        for ci in range(KC):
                            nc.sync.dma_start(out=w_cur[:, ci, :],
                                              in_=src[:, ci, :])
                    xt_src = xt_vid if vid else xt_txt
                    col0 = pair * 256 if vid else (pair - NTV // 2) * 256
                    xt_pair = xpool.tile([128, KC, 256], BF16)
                    nc.sync.dma_start(
                        out=xt_pair[:],
                        in_=xt_src[:, col0:col0 + 256].rearrange("(kc p) t -> p kc t", p=128))

                    for tt in range(2):
                        t = 2 * pair + tt
                        s, ktl = _t_to_skt(t)
                        ps = qkvps.tile([128, 6 * 128], F32)
                        for ci in range(KC):
                            lhs = xt_pair[:, ci, tt * 128:(tt + 1) * 128]
                            nc.tensor.matmul(ps[:, 0:512], lhs, w_cur[:, ci, 0:512],
                                             start=(ci == 0), stop=(ci == KC - 1))
                            nc.tensor.matmul(ps[:, 512:768], lhs, w_cur[:, ci, 512:768],
                                             start=(ci == 0), stop=(ci == KC - 1))

                        # RMSNorm stats for q0,q1,k0,k1 (batched over the 4 blocks)
                        sqf = stat.tile([128, 512], F32, tag="sqf")
                        nc.scalar.activation(sqf[:], ps[:, 0:512], AF.Square)
                        ssum = stat.tile([128, 4], F32, tag="ssum")
                        nc.vector.tensor_reduce(
                            ssum[:], sqf[:].rearrange("p (b f) -> p b f", b=4),
                            mybir.AxisListType.X, mybir.AluOpType.add)
                        rms = stat.tile([128, 4], F32, tag="rms")
                        nc.scalar.activation(rms[:], ssum[:], AF.Sqrt,
                                             bias=eps_sb[:], scale=1.0 / D)
                        rinv = stat.tile([128, 4], F32, tag="rinv")
                        nc.vector.reciprocal(rinv[:], rms[:])
                        rinv_b = rinv[:, :, None].to_broadcast([128, 4, 128])

                        qkbf = qkbfp.tile([128, 512], BF16)
                        ps4 = ps[:, 0:512].rearrange("p (b f) -> p b f", b=4)
                        if vid:
                            qn = qnp.tile([128, 512], F32)
                            qn4 = qn[:].rearrange("p (b f) -> p b f", b=4)
                            nc.vector.tensor_tensor(qn4, ps4, rinv_b,
                                                    mybir.AluOpType.mult)
                            tt_pos = t % 16
                            # views [128, qk, h, d]
                            xv = qn[:].rearrange("p (a h f) -> p a h f", a=2, h=2)
                            qv = qkbf[:].rearrange("p (a h f) -> p a h f", a=2, h=2)
                            x1, x2 = xv[:, :, :, 0:64], xv[:, :, :, 64:128]
                            T = [tab_sb[:, i, :, tt_pos, :].unsqueeze(2)
                                 .to_broadcast([128, 2, 2, 64]) for i in range(4)]
                            tm1 = ropep.tile([128, 2, 2, 64], F32, tag="tm1")
                            tm2 = ropep.tile([128, 2, 2, 64], F32, tag="tm2")
                            nc.vector.tensor_mul(tm1[:], x1, T[0])
                            nc.vector.tensor_mul(tm2[:], x2, T[1])
                            nc.vector.tensor_sub(qv[:, :, :, 0:64], tm1[:], tm2[:])
                            tm3 = ropep.tile([128, 2, 2, 64], F32, tag="tm3")
                            tm4 = ropep.tile([128, 2, 2, 64], F32, tag="tm4")
                            nc.vector.tensor_mul(tm3[:], x1, T[2])
                            nc.vector.tensor_mul(tm4[:], x2, T[3])
                            nc.vector.tensor_add(qv[:, :, :, 64:128], tm3[:], tm4[:])
                        else:
                            qk4 = qkbf[:].rearrange("p (b f) -> p b f", b=4)
                            nc.vector.tensor_tensor(qk4, ps4, rinv_b,
                                                    mybir.AluOpType.mult)
                            for qk in range(2):
                                nc.vector.tensor_mul(qkbf[:, qk * 256:(qk + 1) * 256],
                                                     qkbf[:, qk * 256:(qk + 1) * 256],
                                                     g_sb[:, qk, :])

                        nc.scalar.copy(v_sb[:, t, :], ps[:, 512:768])
                        pending.append((qkbf, s, ktl))
                        if len(pending) > 2:
                            emit_transposes(*pending.pop(0))

                while pending:
                    emit_transposes(*pending.pop(0))

            # ------- Phase 2+3: attention + output proj, one pipeline ------
            with ExitStack() as p2:
                sps = p2.enter_context(tc.tile_pool(name="sps", bufs=2, space="PSUM"))
                dps = p2.enter_context(tc.tile_pool(name="dps", bufs=1, space="PSUM"))
                ops_ = p2.enter_context(tc.tile_pool(name="ops", bufs=2, space="PSUM"))
                yps = p2.enter_context(tc.tile_pool(name="yps", bufs=1, space="PSUM"))
                epool = p2.enter_context(tc.tile_pool(name="e", bufs=4))
                rpool = p2.enter_context(tc.tile_pool(name="rec", bufs=2))
                bpool = p2.enter_context(tc.tile_pool(name="rbc", bufs=2))
                ypool = p2.enter_context(tc.tile_pool(name="ysb", bufs=4))

                def emit_scores(s, h, qs, qn_):
                    E = epool.tile([128, NKT, 512], BF16, tag="E")
                    for j in range((NKT + 1) // 2):
                        k0 = 2 * j
                        kn = 2 if k0 + 1 < NKT else 1
                        ps_s = sps.tile([128, 1024], F32)
                        for u in range(kn):
                            nc.tensor.matmul(
                                ps_s[:, u * 512:u * 512 + qn_],
                                kT[:, h, s, (k0 + u) * 128:(k0 + u + 1) * 128],
                                qT[:, h, s, qs:qs + qn_],
                                start=True, stop=True)
                        if kn == 2 and qn_ == 512:
                            nc.scalar.activation(
                                E[:, k0:k0 + 2, :].rearrange("p a b -> p (a b)"),
                                ps_s[:], AF.Exp, scale=SCALE)
                        else:
                            for u in range(kn):
                                nc.scalar.activation(
                                    E[:, k0 + u, :qn_],
                                    ps_s[:, u * 512:u * 512 + qn_],
                                    AF.Exp, scale=SCALE)
                    return E

                def emit_tail(s, h, qs, qn_, E):
                    ps_d = dps.tile([1, 512], F32)
                    for kt in range(NKT):
                        nc.tensor.matmul(ps_d[:, :qn_], ones_sb[:],
                                         E[:, kt, :qn_],
                                         start=(kt == 0), stop=(kt == NKT - 1))
                    rec = rpool.tile([1, 512], F32)
                    nc.vector.reciprocal(rec[:, :qn_], ps_d[:, :qn_])
                    rbc = bpool.tile([128, 512], F32)
                    nc.gpsimd.partition_broadcast(rbc[:, :qn_], rec[:, :qn_])
                    ps_o = ops_.tile([128, 512], F32)
                    for kt in range(NKT):
                        vt = s * 16 + kt if kt < 16 else NTV + s
                        nc.tensor.matmul(
                            ps_o[:, :qn_],
                            v_sb[:, vt, h * 128:(h + 1) * 128],
                            E[:, kt, :qn_],
                            start=(kt == 0), stop=(kt == NKT - 1))
                    nc.vector.tensor_mul(oT[:, h, s, qs:qs + qn_],
                                         ps_o[:, :qn_], rbc[:, :qn_])

                def emit_y(s, qs, qn_):
                    for ltl in range(qs // 128, (qs + qn_) // 128):
                        t = s * 16 + ltl if ltl < 16 else NTV + s
                        wout_sb = wout_sb_v if t < NTV else wout_sb_t
                        for nb in range(4):
                            ps_y = yps.tile([128, 512], F32)
                            for hh in range(HPC):
                                nc.tensor.matmul(
                                    ps_y[:],
                                    oT[:, hh, s, ltl * 128:(ltl + 1) * 128],
                                    wout_sb[:, hh, nb * 512:(nb + 1) * 512],
                                    start=(hh == 0), stop=(hh == HPC - 1))
                            yt = ypool.tile([128, 512], F32)
                            if (t * 4 + nb) % 2 == 0:
                                nc.scalar.copy(yt[:], ps_y[:])
                            else:
                                nc.vector.tensor_copy(yt[:], ps_y[:])
                            nc.sync.dma_start(
                                out=y[t * 128:(t + 1) * 128,
                                      nb * 512:(nb + 1) * 512],
                                in_=yt[:])

                items = [(s, h, qs, qn_) for s in range(B)
                         for (qs, qn_) in QBS for h in range(HPC)]
                prev = None
                pending_y = []
                for it in items:
                    E = emit_scores(*it)
                    if prev is not None:
                        (ps_, ph_, pqs_, pqn_), pE = prev
                        emit_tail(ps_, ph_, pqs_, pqn_, pE)
                        if ph_ == HPC - 1:
                            pending_y.append((ps_, pqs_, pqn_))
                    near_end = it[2] + it[3] >= L and it[0] == B - 1
                    if len(pending_y) > (0 if near_end else 1):
                        emit_y(*pending_y.pop(0))
                    prev = (it, E)
                (ps_, ph_, pqs_, pqn_), pE = prev
                emit_tail(ps_, ph_, pqs_, pqn_, pE)
                pending_y.append((ps_, pqs_, pqn_))
                while pending_y:
                    emit_y(*pending_y.pop(0))

    nc.compile()
    return nc


_NC_CACHE = None


def _get_nc():
    global _NC_CACHE
    if _NC_CACHE is None:
        _NC_CACHE = build_nc()
    return _NC_CACHE


def _bf16(x):
    return np.ascontiguousarray(x).astype(ml_dtypes.bfloat16)


def kernel(vid, txt, vid_shape, txt_shape,
           Wqkv_vid, Wqkv_txt, gq_vid, gq_txt, gk_vid, gk_txt,
           Wout_vid, Wout_txt, bout_vid, bout_txt):
    global LAST_RESULT
    vid = np.asarray(vid, np.float32)
    txt = np.asarray(txt, np.float32)
    Wqkv_vid = np.asarray(Wqkv_vid, np.float32)
    Wqkv_txt = np.asarray(Wqkv_txt, np.float32)
    Wout_vid = np.asarray(Wout_vid, np.float32)
    Wout_txt = np.asarray(Wout_txt, np.float32)
    gq_vid = np.asarray(gq_vid, np.float32)
    gq_txt = np.asarray(gq_txt, np.float32)
    gk_vid = np.asarray(gk_vid, np.float32)
    gk_txt = np.asarray(gk_txt, np.float32)

    nc = _get_nc()

    xt_vid = _bf16(vid.T)
    xt_txt = _bf16(txt.T)

    # rope tables with g folded in
    pos = np.arange(LV, dtype=np.float32)
    inv_freq = 1.0 / (ROPE_BASE ** (np.arange(64, dtype=np.float32) / 64.0))
    ang = pos[:, None] * inv_freq[None, :]
    cos, sin = np.cos(ang), np.sin(ang)
    # tabs[i, qk]: rope factor i for {q, k}; out1 = x1*T0 - x2*T1,
    # out2 = x1*T2 + x2*T3 (g folded in)
    tabs = np.stack([
        np.stack([cos * gq_vid[None, :64], cos * gk_vid[None, :64]]),
        np.stack([sin * gq_vid[None, 64:], sin * gk_vid[None, 64:]]),
        np.stack([sin * gq_vid[None, :64], sin * gk_vid[None, :64]]),
        np.stack([cos * gq_vid[None, 64:], cos * gk_vid[None, 64:]]),
    ])
    tabs = _bf16(tabs)
    g_txt = _bf16(np.stack([
        np.tile(gq_txt, (128, HPC)),
        np.tile(gk_txt, (128, HPC)),
    ]))

    in_maps = []
    for c in range(NCORES):
        h0, h1 = HPC * c, HPC * c + 1
        def _wqkv(W):
            Wq, Wk, Wv = W[:, :INNER], W[:, INNER:2 * INNER], W[:, 2 * INNER:]
            cols = []
            for Wx in (Wq, Wk, Wv):
                cols.append(Wx[:, h0 * D:(h0 + 1) * D])
                cols.append(Wx[:, h1 * D:(h1 + 1) * D])
            return _bf16(np.concatenate(cols, axis=1))
        def _wout(W):
            return _bf16(np.concatenate(
                [W[h0 * D:(h0 + 1) * D, :], W[h1 * D:(h1 + 1) * D, :]], axis=0))
        in_maps.append({
            "xt_vid": xt_vid, "xt_txt": xt_txt,
            "wqkv_vid": _wqkv(Wqkv_vid), "wqkv_txt": _wqkv(Wqkv_txt),
            "wout_vid": _wout(Wout_vid), "wout_txt": _wout(Wout_txt),
            "tabs": tabs, "g_txt": g_txt,
        })

    try:
        res = run_bass_kernel_spmd(nc, in_maps, list(range(NCORES)))
    except Exception:
        # transient device state (e.g. NRT exec-unit wedge) — one retry
        res = run_bass_kernel_spmd(nc, in_maps, list(range(NCORES)))
    LAST_RESULT = res
    Y = np.zeros((NTOK, DIM), np.float32)
    for c in range(NCORES):
        Y += res.results[c]["y"]
    vid_out = Y[:B * LV] + np.asarray(bout_vid, np.float32)[None, :]
    txt_out = Y[B * LV:] + np.asarray(bout_txt, np.float32)[None, :]
    return vid_out, txt_out


# revision 35
# speedup vs baseline: 1.1494x; 1.0879x over previous
"""NaMMAttention (multimodal video+text attention) on 8 Trainium2 cores.

Sharding: tensor-parallel over heads (16 heads -> 2 per core). Each core:
  - computes QKV projections for its 2 heads over the full sequence
    (token-major), applies QK-RMSNorm + RoPE (vid only),
  - transposes Q/K to d-major, runs attention in the S^T layout
    (softmax reduction via ones-matmul, exp without max subtraction --
    scores are bounded by |q||k|/sqrt(D) <= sqrt(128) after RMSNorm),
  - computes its partial output projection (rows of Wout owned by its
    heads) and writes a full-shape fp32 partial.
Host sums the 8 partials and adds biases.
"""

import numpy as np
import ml_dtypes

import concourse.bass as bass
import concourse.mybir as mybir
import concourse.tile as tile
from concourse import bacc
from concourse.bass_utils import run_bass_kernel_spmd
from concourse.masks import make_identity

F32 = mybir.dt.float32
BF16 = mybir.dt.bfloat16
AF = mybir.ActivationFunctionType

B, LV, LT = 2, 2048, 128
H, D = 16, 128
DIM = 2048          # VID_DIM == TXT_DIM
INNER = H * D
NCORES = 8
HPC = H // NCORES   # heads per core = 2
EPS = 1e-6
ROPE_BASE = 10000.0
SCALE = D ** -0.5

KC = DIM // 128     # 16 contraction tiles
NTV = B * LV // 128  # 32 vid token tiles
NTT = B * LT // 128  # 2 txt token tiles
NT = NTV + NTT       # 34
L = LV + LT          # 2176 per-sample seq
NKT = L // 128       # 17 k tiles per sample
NTOK = B * L         # 4352 rows of y
QBS = [(0, 512), (512, 512), (1024, 512), (1536, 512), (2048, 128)]

LAST_RESULT = None  # test harness reads exec_time_ns off this


def _t_to_skt(t):
    """token tile index -> (sample, local k-tile)"""
    if t < NTV:
        return t // 16, t % 16
    return t - NTV, 16


def build_nc():
    nc = bacc.Bacc("TRN2", target_bir_lowering=False, debug=False,
                   num_devices=NCORES)

    xt_vid = nc.dram_tensor("xt_vid", [DIM, B * LV], BF16, kind="ExternalInput").ap()
    xt_txt = nc.dram_tensor("xt_txt", [DIM, B * LT], BF16, kind="ExternalInput").ap()
    wqkv_vid = nc.dram_tensor("wqkv_vid", [DIM, 6 * 128], BF16, kind="ExternalInput").ap()
    wqkv_txt = nc.dram_tensor("wqkv_txt", [DIM, 6 * 128], BF16, kind="ExternalInput").ap()
    wout_vid = nc.dram_tensor("wout_vid", [HPC * D, DIM], BF16, kind="ExternalInput").ap()
    wout_txt = nc.dram_tensor("wout_txt", [HPC * D, DIM], BF16, kind="ExternalInput").ap()
    # rope tables with gq/gk folded in: order
    # [cosA_q, sinB_q, sinA_q, cosB_q, cosA_k, sinB_k, sinA_k, cosB_k]
    tabs = nc.dram_tensor("tabs", [4, 2, LV, 64], BF16, kind="ExternalInput").ap()
    g_txt = nc.dram_tensor("g_txt", [2, 128, HPC * D], BF16, kind="ExternalInput").ap()
    y = nc.dram_tensor("y", [NTOK, DIM], F32, kind="ExternalOutput").ap()

    with tile.TileContext(nc) as tc:
        from contextlib import ExitStack
        with ExitStack() as ctx:
            persist = ctx.enter_context(tc.tile_pool(name="persist", bufs=1))
            qT = persist.tile([128, HPC, B, L], BF16, tag="qT")
            kT = persist.tile([128, HPC, B, L], BF16, tag="kT")
            v_sb = persist.tile([128, NT, HPC * D], BF16, tag="v")
            oT = persist.tile([128, HPC, B, L], BF16, tag="oT")
            wout_sb_v = persist.tile([128, HPC, DIM], BF16, tag="wout_v")
            wout_sb_t = persist.tile([128, HPC, DIM], BF16, tag="wout_t")
            tab_sb = persist.tile([128, 4, 2, 16, 64], BF16, tag="tabs")
            g_sb = persist.tile([128, 2, HPC * D], BF16, tag="g_txt")
            ones_sb = persist.tile([128, 1], BF16, tag="ones")
            ident = persist.tile([128, 128], BF16, tag="ident")
            eps_sb = persist.tile([128, 1], F32, tag="eps")
            nc.gpsimd.memset(eps_sb[:], EPS)

            # off the critical sync-DMA queue: phase 1's first W/x loads go first
            nc.scalar.dma_start(out=wout_sb_v[:], in_=wout_vid.rearrange("(h p) n -> p h n", p=128))
            nc.scalar.dma_start(out=wout_sb_t[:], in_=wout_txt.rearrange("(h p) n -> p h n", p=128))
            nc.scalar.dma_start(out=tab_sb[:], in_=tabs.rearrange("i j (t p) f -> p i j t f", p=128))
            nc.scalar.dma_start(out=g_sb[:], in_=g_txt.rearrange("i p n -> p i n"))
            nc.gpsimd.memset(ones_sb[:], 1.0)
            make_identity(nc, ident[:])

            # ---------------- Phase 1: QKV + norm + rope + transpose -------
            with ExitStack() as p1:
                wpool = p1.enter_context(tc.tile_pool(name="wqkv", bufs=1))
                xpool = p1.enter_context(tc.tile_pool(name="xt", bufs=3))
                qkvps = p1.enter_context(tc.tile_pool(name="qkvps", bufs=3, space="PSUM"))
                trps = p1.enter_context(tc.tile_pool(name="trps", bufs=2, space="PSUM"))
                stat = p1.enter_context(tc.tile_pool(name="stat", bufs=3))
                qnp = p1.enter_context(tc.tile_pool(name="qn", bufs=2))
                qkbfp = p1.enter_context(tc.tile_pool(name="qkbf", bufs=4))
                ropep = p1.enter_context(tc.tile_pool(name="rope", bufs=3))

                pending = []

                def emit_transposes(qkbf, s, ktl):
                    for b in range(4):
                        h = b % 2
                        dst = qT if b < 2 else kT
                        pst = trps.tile([128, 128], BF16)
                        nc.tensor.transpose(pst[:], qkbf[:, b * 128:(b + 1) * 128],
                                            ident[:])
                        nc.scalar.copy(
                            dst[:, h, s, ktl * 128:(ktl + 1) * 128], pst[:])

                w_cur = None
                # txt pair first: attention k-tile 16 (txt) is needed by the
                # very first phase-2 item, so produce it early.
                pair_order = [NTV // 2] + list(range(NTV // 2))
                for pi, pair in enumerate(pair_order):
                    vid = pair < NTV // 2
                    if pi < 2:
                        w_cur = wpool.tile([128, KC, 6 * 128], BF16, tag="w")
                        src = (wqkv_vid if vid else wqkv_txt).rearrange(
                            "(kc p) n -> p kc n", p=128)
                        # per-kc chunks so the first matmuls start after 150KB,
                        # not after the whole 2.4MB weight load
                        for ci in range(KC):
                            nc.sync.dma_start(out=w_cur[:, ci, :],
                                              in_=src[:, ci, :])
                    xt_src = xt_vid if vid else xt_txt
                    col0 = pair * 256 if vid else (pair - NTV // 2) * 256
                    xt_pair = xpool.tile([128, KC, 256], BF16)
                    nc.sync.dma_start(
                        out=xt_pair[:],
                        in_=xt_src[:, col0:col0 + 256].rearrange("(kc p) t -> p kc t", p=128))

                    for tt in range(2):
                        t = 2 * pair + tt
                        s, ktl = _t_to_skt(t)
                        ps = qkvps.tile([128, 6 * 128], F32)
                        for ci in range(KC):
                            lhs = xt_pair[:, ci, tt * 128:(tt + 1) * 128]
                            nc.tensor.matmul(ps[:, 0:512], lhs, w_cur[:, ci, 0:512],
                                             start=(ci == 0), stop=(ci == KC - 1))
                            nc.tensor.matmul(ps[:, 512:768], lhs, w_cur[:, ci, 512:768],
                                             start=(ci == 0), stop=(ci == KC - 1))

                        # RMSNorm stats for q0,q1,k0,k1 (batched over the 4 blocks)
                        sqf = stat.tile([128, 512], F32, tag="sqf")
                        nc.scalar.activation(sqf[:], ps[:, 0:512], AF.Square)
                        ssum = stat.tile([128, 4], F32, tag="ssum")
                        nc.vector.tensor_reduce(
                            ssum[:], sqf[:].rearrange("p (b f) -> p b f", b=4),
                            mybir.AxisListType.X, mybir.AluOpType.add)
                        rms = stat.tile([128, 4], F32, tag="rms")
                        nc.scalar.activation(rms[:], ssum[:], AF.Sqrt,
                                             bias=eps_sb[:], scale=1.0 / D)
                        rinv = stat.tile([128, 4], F32, tag="rinv")
                        nc.vector.reciprocal(rinv[:], rms[:])
                        rinv_b = rinv[:, :, None].to_broadcast([128, 4, 128])

                        qkbf = qkbfp.tile([128, 512], BF16)
                        ps4 = ps[:, 0:512].rearrange("p (b f) -> p b f", b=4)
                        if vid:
                            qn = qnp.tile([128, 512], F32)
                            qn4 = qn[:].rearrange("p (b f) -> p b f", b=4)
                            nc.vector.tensor_tensor(qn4, ps4, rinv_b,
                                                    mybir.AluOpType.mult)
                            tt_pos = t % 16
                            # views [128, qk, h, d]
                            xv = qn[:].rearrange("p (a h f) -> p a h f", a=2, h=2)
                            qv = qkbf[:].rearrange("p (a h f) -> p a h f", a=2, h=2)
                            x1, x2 = xv[:, :, :, 0:64], xv[:, :, :, 64:128]
                            T = [tab_sb[:, i, :, tt_pos, :].unsqueeze(2)
                                 .to_broadcast([128, 2, 2, 64]) for i in range(4)]
                            tm1 = ropep.tile([128, 2, 2, 64], F32, tag="tm1")
                            tm2 = ropep.tile([128, 2, 2, 64], F32, tag="tm2")
                            nc.vector.tensor_mul(tm1[:], x1, T[0])
                            nc.vector.tensor_mul(tm2[:], x2, T[1])
                            nc.vector.tensor_sub(qv[:, :, :, 0:64], tm1[:], tm2[:])
                            tm3 = ropep.tile([128, 2, 2, 64], F32, tag="tm3")
                            tm4 = ropep.tile([128, 2, 2, 64], F32, tag="tm4")
                            nc.vector.tensor_mul(tm3[:], x1, T[2])
                            nc.vector.tensor_mul(tm4[:], x2, T[3])
                            nc.vector.tensor_add(qv[:, :, :, 64:128], tm3[:], tm4[:])
                        else:
                            qk4 = qkbf[:].rearrange("p (b f) -> p b f", b=4)
                            nc.vector.tensor_tensor(qk4, ps4, rinv_b,
                                                    mybir.AluOpType.mult)
                            for qk in range(2):
                                nc.vector.tensor_mul(qkbf[:, qk * 256:(qk + 1) * 256],
                                                     qkbf[:, qk * 256:(qk + 1) * 256],
                                                     g_sb[:, qk, :])

                        nc.scalar.copy(v_sb[:, t, :], ps[:, 512:768])
                        pending.append((qkbf, s, ktl))
                        if len(pending) > 2:
                            emit_transposes(*pending.pop(0))

                while pending:
                    emit_transposes(*pending.pop(0))

            # ------- Phase 2+3: attention + output proj, one pipeline ------
            with ExitStack() as p2:
                sps = p2.enter_context(tc.tile_pool(name="sps", bufs=2, space="PSUM"))
                dps = p2.enter_context(tc.tile_pool(name="dps", bufs=1, space="PSUM"))
                ops_ = p2.enter_context(tc.tile_pool(name="ops", bufs=1, space="PSUM"))
                yps = p2.enter_context(tc.tile_pool(name="yps", bufs=2, space="PSUM"))
                epool = p2.enter_context(tc.tile_pool(name="e", bufs=4))
                rpool = p2.enter_context(tc.tile_pool(name="rec", bufs=2))
                bpool = p2.enter_context(tc.tile_pool(name="rbc", bufs=2))
                ypool = p2.enter_context(tc.tile_pool(name="ysb", bufs=6))

                def emit_scores(s, h, qs, qn_):
                    E = epool.tile([128, NKT, 512], BF16, tag="E")
                    for j in range((NKT + 1) // 2):
                        k0 = 2 * j
                        kn = 2 if k0 + 1 < NKT else 1
                        ps_s = sps.tile([128, 1024], F32)
                        for u in range(kn):
                            nc.tensor.matmul(
                                ps_s[:, u * 512:u * 512 + qn_],
                                kT[:, h, s, (k0 + u) * 128:(k0 + u + 1) * 128],
                                qT[:, h, s, qs:qs + qn_],
                                start=True, stop=True)
                        if kn == 2 and qn_ == 512:
                            nc.scalar.activation(
                                E[:, k0:k0 + 2, :].rearrange("p a b -> p (a b)"),
                                ps_s[:], AF.Exp, scale=SCALE)
                        else:
                            for u in range(kn):
                                nc.scalar.activation(
                                    E[:, k0 + u, :qn_],
                                    ps_s[:, u * 512:u * 512 + qn_],
                                    AF.Exp, scale=SCALE)
                    return E

                def emit_tail(s, h, qs, qn_, E):
                    ps_d = dps.tile([1, 512], F32)
                    for kt in range(NKT):
                        nc.tensor.matmul(ps_d[:, :qn_], ones_sb[:],
                                         E[:, kt, :qn_],
                                         start=(kt == 0), stop=(kt == NKT - 1))
                    rec = rpool.tile([1, 512], F32)
                    nc.vector.reciprocal(rec[:, :qn_], ps_d[:, :qn_])
                    rbc = bpool.tile([128, 512], F32)
                    nc.gpsimd.partition_broadcast(rbc[:, :qn_], rec[:, :qn_])
                    ps_o = ops_.tile([128, 512], F32)
                    for kt in range(NKT):
                        vt = s * 16 + kt if kt < 16 else NTV + s
                        nc.tensor.matmul(
                            ps_o[:, :qn_],
                            v_sb[:, vt, h * 128:(h + 1) * 128],
                            E[:, kt, :qn_],
                            start=(kt == 0), stop=(kt == NKT - 1))
                    nc.vector.tensor_mul(oT[:, h, s, qs:qs + qn_],
                                         ps_o[:, :qn_], rbc[:, :qn_])

                def emit_y(s, qs, qn_):
                    for ltl in range(qs // 128, (qs + qn_) // 128):
                        t = s * 16 + ltl if ltl < 16 else NTV + s
                        wout_sb = wout_sb_v if t < NTV else wout_sb_t
                        for nb in range(4):
                            ps_y = yps.tile([128, 512], F32)
                            for hh in range(HPC):
                                nc.tensor.matmul(
                                    ps_y[:],
                                    oT[:, hh, s, ltl * 128:(ltl + 1) * 128],
                                    wout_sb[:, hh, nb * 512:(nb + 1) * 512],
                                    start=(hh == 0), stop=(hh == HPC - 1))
                            yt = ypool.tile([128, 512], F32)
                            if (t * 4 + nb) % 2 == 0:
                                nc.scalar.copy(yt[:], ps_y[:])
                            else:
                                nc.vector.tensor_copy(yt[:], ps_y[:])
                            nc.sync.dma_start(
                                out=y[t * 128:(t + 1) * 128,
                                      nb * 512:(nb + 1) * 512],
                                in_=yt[:])

                items = [(s, h, qs, qn_) for s in range(B)
                         for (qs, qn_) in QBS for h in range(HPC)]
                prev = None
                pending_y = []
                for it in items:
                    E = emit_scores(*it)
                    if prev is not None:
                        (ps_, ph_, pqs_, pqn_), pE = prev
                        emit_tail(ps_, ph_, pqs_, pqn_, pE)
                        if ph_ == HPC - 1:
                            pending_y.append((ps_, pqs_, pqn_))
                    near_end = it[2] + it[3] >= L and it[0] == B - 1
                    if len(pending_y) > (0 if near_end else 1):
                        emit_y(*pending_y.pop(0))
                    prev = (it, E)
                (ps_, ph_, pqs_, pqn_), pE = prev
                emit_tail(ps_, ph_, pqs_, pqn_, pE)
                pending_y.append((ps_, pqs_, pqn_))
                while pending_y:
                    emit_y(*pending_y.pop(0))

    nc.compile()
    return nc


_NC_CACHE = None


def _get_nc():
    global _NC_CACHE
    if _NC_CACHE is None:
        _NC_CACHE = build_nc()
    return _NC_CACHE


def _bf16(x):
    return np.ascontiguousarray(x).astype(ml_dtypes.bfloat16)


def kernel(vid, txt, vid_shape, txt_shape,
           Wqkv_vid, Wqkv_txt, gq_vid, gq_txt, gk_vid, gk_txt,
           Wout_vid, Wout_txt, bout_vid, bout_txt):
    global LAST_RESULT
    vid = np.asarray(vid, np.float32)
    txt = np.asarray(txt, np.float32)
    Wqkv_vid = np.asarray(Wqkv_vid, np.float32)
    Wqkv_txt = np.asarray(Wqkv_txt, np.float32)
    Wout_vid = np.asarray(Wout_vid, np.float32)
    Wout_txt = np.asarray(Wout_txt, np.float32)
    gq_vid = np.asarray(gq_vid, np.float32)
    gq_txt = np.asarray(gq_txt, np.float32)
    gk_vid = np.asarray(gk_vid, np.float32)
    gk_txt = np.asarray(gk_txt, np.float32)

    nc = _get_nc()

    xt_vid = _bf16(vid.T)
    xt_txt = _bf16(txt.T)

    # rope tables with g folded in
    pos = np.arange(LV, dtype=np.float32)
    inv_freq = 1.0 / (ROPE_BASE ** (np.arange(64, dtype=np.float32) / 64.0))
    ang = pos[:, None] * inv_freq[None, :]
    cos, sin = np.cos(ang), np.sin(ang)
    # tabs[i, qk]: rope factor i for {q, k}; out1 = x1*T0 - x2*T1,
    # out2 = x1*T2 + x2*T3 (g folded in)
    tabs = np.stack([
        np.stack([cos * gq_vid[None, :64], cos * gk_vid[None, :64]]),
        np.stack([sin * gq_vid[None, 64:], sin * gk_vid[None, 64:]]),
        np.stack([sin * gq_vid[None, :64], sin * gk_vid[None, :64]]),
        np.stack([cos * gq_vid[None, 64:], cos * gk_vid[None, 64:]]),
    ])
    tabs = _bf16(tabs)
    g_txt = _bf16(np.stack([
        np.tile(gq_txt, (128, HPC)),
        np.tile(gk_txt, (128, HPC)),
    ]))

    in_maps = []
    for c in range(NCORES):
        h0, h1 = HPC * c, HPC * c + 1
        def _wqkv(W):
            Wq, Wk, Wv = W[:, :INNER], W[:, INNER:2 * INNER], W[:, 2 * INNER:]
            cols = []
            for Wx in (Wq, Wk, Wv):
                cols.append(Wx[:, h0 * D:(h0 + 1) * D])
                cols.append(Wx[:, h1 * D:(h1 + 1) * D])
            return _bf16(np.concatenate(cols, axis=1))
        def _wout(W):
            return _bf16(np.concatenate(
                [W[h0 * D:(h0 + 1) * D, :], W[h1 * D:(h1 + 1) * D, :]], axis=0))
        in_maps.append({
            "xt_vid": xt_vid, "xt_txt": xt_txt,
            "wqkv_vid": _wqkv(Wqkv_vid), "wqkv_txt": _wqkv(Wqkv_txt),
            "wout_vid": _wout(Wout_vid), "wout_txt": _wout(Wout_txt),
            "tabs": tabs, "g_txt": g_txt,
        })

    try:
        res = run_bass_kernel_spmd(nc, in_maps, list(range(NCORES)))
    except Exception:
        # transient device state (e.g. NRT exec-unit wedge) — one retry
        res = run_bass_kernel_spmd(nc, in_maps, list(range(NCORES)))
    LAST_RESULT = res
    Y = np.zeros((NTOK, DIM), np.float32)
    for c in range(NCORES):
        Y += res.results[c]["y"]
    vid_out = Y[:B * LV] + np.asarray(bout_vid, np.float32)[None, :]
    txt_out = Y[B * LV:] + np.asarray(bout_txt, np.float32)[None, :]
    return vid_out, txt_out
